# revision 12
# baseline (speedup 1.0000x reference)
"""CrossAttentionBlockLLaMA on 8 Trainium2 NeuronCores (Bass/Tile).

Sharding:
  - QKV + attention: tensor-parallel over heads (2 heads/core).
  - Output projection wo: row-sharded over heads; each core computes a
    partial h for ALL tokens, written window-major [8, D, TC]; a
    ReduceScatter sums partials and hands core r exactly h.T[:, tokens_r].
  - FFN + post-norm: token-parallel (TC tokens/core), full weights.

Perf structure (v2):
  - q/k/v and attention output o live entirely in SBUF (no DRAM
    round-trip), so attention + wo need no input DMA and keep the PE
    fed while the ReduceScatter of the other stream is on the wire.
  - Row-sums (softmax denominators, RMS mean-squares) accumulate on the
    Vector engine via f16 tile adds; only ONE ones-matmul per reduction
    hits the PE instead of 16.
  - Residual is read f16 straight from the pre-transposed activations.

Layouts: host pre-transposes activations/weights so every matmul's
contraction dim is on SBUF partitions. attn_norm_w and 1/sqrt(HD) are
folded into wq/wk/wv host-side; per-token 1/rms factors are applied to
q/k/v on device. Matmul inputs fp16, PSUM accumulation fp32.

Self-contained: hardcodes shapes from the problem spec.
"""
import numpy as np

NCORES = 8
EPS = 1e-5


class Cfg:
    def __init__(self, B=2, S=2048, D=2048, H=16, HD=128, FF=5632):
        self.B, self.S, self.D, self.H, self.HD, self.FF = B, S, D, H, HD, FF
        self.T = B * S                    # total tokens
        self.TC = self.T // NCORES        # tokens per core (phase 3)
        self.NQ = (H // NCORES) * HD      # per-core head dims
        self.DT = D // 128                # d-tiles
        self.FT = FF // 128               # ff-tiles
        self.NQT = self.NQ // 128         # per-core head-dim tiles
        self.TCH = min(512, self.T)       # phase-1 token chunk
        self.QCH = min(512, S)            # phase-2 query chunk
        self.TCW = min(512, self.TC)      # phase-3 / wo token chunk
        assert self.T % self.TCH == 0 and S % self.QCH == 0
        assert self.TC % self.TCW == 0 and S % 128 == 0
        assert HD == 128 and D % 128 == 0 and FF % 128 == 0


FULL = Cfg()


def build(cfg=FULL):
    import concourse.mybir as mybir
    import concourse.tile as tile
    from concourse import bacc

    F16 = mybir.dt.float16
    F32 = mybir.dt.float32

    c = cfg
    nc = bacc.Bacc("TRN2", target_bir_lowering=False, debug=False,
                   num_devices=NCORES)

    ins = {}
    outs = {}
    for s in ("x", "y"):
        ins[f"{s}T"] = nc.dram_tensor(f"{s}T", [c.D, c.T], F16,
                                      kind="ExternalInput").ap()
        for w in ("wq", "wk", "wv"):
            ins[f"{w}T_{s}"] = nc.dram_tensor(
                f"{w}T_{s}", [c.D, c.NQ], F16, kind="ExternalInput").ap()
        ins[f"woT_{s}"] = nc.dram_tensor(
            f"woT_{s}", [c.NQ, c.D], F16, kind="ExternalInput").ap()
        ins[f"w1T_{s}"] = nc.dram_tensor(
            f"w1T_{s}", [c.D, c.FF], F16, kind="ExternalInput").ap()
        ins[f"w3T_{s}"] = nc.dram_tensor(
            f"w3T_{s}", [c.D, c.FF], F16, kind="ExternalInput").ap()
        ins[f"w2T_{s}"] = nc.dram_tensor(
            f"w2T_{s}", [c.FF, c.D], F16, kind="ExternalInput").ap()
        ins[f"res_{s}"] = nc.dram_tensor(
            f"res_{s}", [c.D, c.TC], F16, kind="ExternalInput").ap()
        ins[f"fnorm_{s}"] = nc.dram_tensor(
            f"fnorm_{s}", [128, c.DT], F32, kind="ExternalInput").ap()
        outs[s] = nc.dram_tensor(f"out_{s}", [c.D, c.TC], F32,
                                 kind="ExternalOutput").ap()

    with tile.TileContext(nc) as tc:
        _emit(tc, nc, c, ins, outs)
    nc.compile()
    return nc


def _emit(tc, nc, c, ins, outs):
    import concourse.mybir as mybir

    F16 = mybir.dt.float16
    F32 = mybir.dt.float32
    AF = mybir.ActivationFunctionType
    one_over_d = 1.0 / c.D

    with (
        tc.tile_pool(name="psum", bufs=1, space="PSUM") as ps,
        tc.tile_pool(name="const", bufs=1) as const,
        tc.tile_pool(name="dram", bufs=1, space="DRAM") as dram,
    ):
        ones_col = const.tile([128, 1], F16)
        nc.vector.memset(ones_col[:], 1.0)
        eps1 = const.tile([1, 1], F32)
        nc.vector.memset(eps1[:], EPS)

        sc = {}
        for s in ("x", "y"):
            sc[f"v_{s}"] = dram.tile([c.T, c.NQ], F16, name=f"v_{s}")
            # wo partials, window-major: [NCORES windows, D, TC]
            sc[f"hp_{s}"] = dram.tile([NCORES * c.D, c.TC], F16,
                                      name=f"hp_{s}")
            sc[f"h_{s}"] = dram.tile([c.D, c.TC], F16, name=f"h_{s}")

        def mm(shape, name):
            return ps.tile(shape, F32, tag="mm", bufs=6, name=name)

        def row(shape, name):
            return ps.tile(shape, F32, tag="row", bufs=2, name=name)

        def bcast_free(rsq16, width, sb_pool, name):
            """[1,width] f16 -> [128,width] f16 via DRAM stride-0 DMA on
            the scalar queue (off the PE queue and the store-heavy sync
            queue)."""
            rd = dram.tile([1, width], F16, tag="bc_row", bufs=4,
                           name=f"bcd_{name}")
            nc.scalar.dma_start(rd[:], rsq16[:1, :width])
            bc16 = sb_pool.tile([128, width], F16, tag="bc16",
                                name=f"bc16_{name}")
            nc.scalar.dma_start(bc16[:], rd[:].to_broadcast((128, width)))
            return bc16

        NFH = c.FT // 2  # ff tiles per phase-3 half-pass

        # small long-lived pool: cross-phase prefetch targets
        with tc.tile_pool(name="pfp", bufs=1) as pfp:
          pf = {}

          def prefetch_p3(s):
            """First-chunk loads for phase 3, issued from the gpsimd queue
            right after the ReduceScatter trigger so they land while the
            PE still works on phase-2 compute."""
            fnorm = pfp.tile([128, c.DT], F32, tag="fnorm", bufs=2,
                             name=f"fnorm_{s}")
            nc.gpsimd.dma_start(fnorm[:], ins[f"fnorm_{s}"])
            pf[s] = {"fnorm": fnorm}
            if s != "x":
                return
            w1 = pfp.tile([128, c.DT, 256], F16, tag="w1c0", name="w1c0")
            nc.gpsimd.dma_start(
                w1[:], ins[f"w1T_{s}"][:, :256]
                .rearrange("(o p) j -> p o j", p=128))
            w3 = pfp.tile([128, c.DT, 256], F16, tag="w3c0", name="w3c0")
            nc.gpsimd.dma_start(
                w3[:], ins[f"w3T_{s}"][:, :256]
                .rearrange("(o p) j -> p o j", p=128))
            pf[s].update(w1=w1, w3=w3)

          with tc.tile_pool(name="qko", bufs=1) as qko:
            # persistent SBUF q/k per stream (8 MB total); v spills to DRAM
            QT, KT = {}, {}
            for s in ("x", "y"):
                QT[s] = qko.tile([128, c.NQT, c.T], F16, name=f"qT_{s}")
                KT[s] = qko.tile([128, c.NQT, c.T], F16, name=f"kT_{s}")

            # ======== PHASE 1: RMSNorm stats + QKV projections ========
            with (
                tc.tile_pool(name="p1w", bufs=1) as p1w,
                tc.tile_pool(name="p1a", bufs=2) as p1a,
                tc.tile_pool(name="p1s", bufs=2) as p1s,
            ):
                W = {}
                for s in ("x", "y"):
                    for w in ("wq", "wk", "wv"):
                        t = p1w.tile([128, c.DT, c.NQ], F16,
                                     name=f"{w}_{s}_sb")
                        nc.scalar.dma_start(
                            t[:],
                            ins[f"{w}T_{s}"].rearrange("(o p) j -> p o j",
                                                       p=128))
                        W[f"{w}{s}"] = t

                for ich in range(c.T // c.TCH):
                    tsl = slice(ich * c.TCH, (ich + 1) * c.TCH)
                    act = {}
                    rsq_free = {}
                    rsq_part = {}
                    for s in ("x", "y"):
                        at = p1a.tile([128, c.DT, c.TCH], F16,
                                      tag=f"act_{s}", name=f"act_{s}")
                        nc.sync.dma_start(
                            at[:],
                            ins[f"{s}T"][:, tsl].rearrange(
                                "(o p) t -> p o t", p=128))
                        act[s] = at

                        # mean-square accumulate on DVE, one PE matmul
                        acc = p1s.tile([128, c.TCH], F16, tag="msacc",
                                       bufs=2, name=f"msacc_{s}")
                        nc.vector.tensor_mul(acc[:], at[:, 0], at[:, 0])
                        for o in range(1, c.DT):
                            sq = p1s.tile([128, c.TCH], F16, tag="sq",
                                          name=f"sq_{s}{o}")
                            nc.vector.tensor_mul(sq[:], at[:, o], at[:, o])
                            nc.vector.tensor_add(acc[:], acc[:], sq[:])
                        ms_ps = row([1, c.TCH], f"ms_{s}")
                        nc.tensor.matmul(ms_ps[:], ones_col[:], acc[:],
                                         start=True, stop=True)
                        rms = p1s.tile([1, c.TCH], F32, tag="rms",
                                       bufs=1, name=f"rms_{s}")
                        nc.scalar.activation(rms[:], ms_ps[:], AF.Sqrt,
                                             bias=eps1[:], scale=one_over_d)
                        rsqf = p1s.tile([1, c.TCH], F32, tag="rsqf",
                                        bufs=1, name=f"rsqf_{s}")
                        nc.vector.reciprocal(rsqf[:], rms[:])
                        rsqf16 = p1s.tile([1, c.TCH], F16, tag="rsqf16",
                                          name=f"rsqf16_{s}")
                        nc.vector.tensor_copy(rsqf16[:], rsqf[:])
                        rsq_free[s] = rsqf16

                        nsub = c.TCH // 128
                        rfd = dram.tile([1, c.TCH], F32, tag="rsq_row",
                                        bufs=4, name=f"rfd_{s}")
                        nc.scalar.dma_start(rfd[:], rsqf[:])
                        rsqT = p1s.tile([128, nsub], F32, tag="rsqT",
                                        name=f"rsqT_{s}")
                        nc.scalar.dma_start(
                            rsqT[:],
                            rfd[0, :].rearrange("(n p) -> p n", p=128))
                        rsq_part[s] = rsqT

                    for s in ("x", "y"):
                        kv = "y" if s == "x" else "x"
                        bc_q = bcast_free(rsq_free[s], c.TCH, p1s,
                                          f"q{s}{ich}")
                        bc_k = bcast_free(rsq_free[kv], c.TCH, p1s,
                                          f"k{s}{ich}")

                        for (wname, src, bc, dst) in (
                            ("wq", s, bc_q, QT[s]),
                            ("wk", kv, bc_k, KT[s]),
                        ):
                            for jt in range(c.NQT):
                                pm = mm([128, c.TCH], f"{wname}{s}{jt}")
                                wt = W[f"{wname}{s}"]
                                for o in range(c.DT):
                                    nc.tensor.matmul(
                                        pm[:],
                                        wt[:, o, jt * 128:(jt + 1) * 128],
                                        act[src][:, o],
                                        start=(o == 0),
                                        stop=(o == c.DT - 1))
                                nc.vector.tensor_mul(dst[:, jt, tsl],
                                                     pm[:], bc[:])

                        for i in range(c.TCH // 128):
                            pv = mm([128, c.NQ], f"v{s}{i}")
                            for o in range(c.DT):
                                nc.tensor.matmul(
                                    pv[:],
                                    act[kv][:, o, i * 128:(i + 1) * 128],
                                    W[f"wv{s}"][:, o, :],
                                    start=(o == 0), stop=(o == c.DT - 1))
                            vt = p1s.tile([128, c.NQ], F16, tag="v_out",
                                          name=f"v{s}{i}o")
                            nc.vector.tensor_scalar_mul(
                                vt[:], pv[:], rsq_part[kv][:, i:i + 1])
                            nc.sync.dma_start(
                                sc[f"v_{s}"][ich * c.TCH + i * 128:
                                             ich * c.TCH + (i + 1) * 128,
                                             :],
                                vt[:])

            # ======== PHASE 2: attention + wo partial + ReduceScatter ====
            with (
                tc.tile_pool(name="p2", bufs=2) as p2,
                tc.tile_pool(name="p2o", bufs=1) as p2o,
                tc.tile_pool(name="p2w", bufs=2) as p2w,
            ):
                nk = c.S // 128
                # hoist ALL attention/wo input loads: nothing below needs
                # the sync DMA queue once the hp stores start flowing
                VT, WO = {}, {}
                for s in ("x", "y"):
                    WO[s] = p2w.tile([128, c.NQT, c.D], F16, tag="wo",
                                     bufs=2, name=f"wo_{s}")
                    nc.sync.dma_start(
                        WO[s][:],
                        ins[f"woT_{s}"].rearrange("(o p) j -> p o j",
                                                  p=128))
                for s in ("x", "y"):
                    for b in range(c.B):
                        for h in range(c.NQT):
                            vt = p2.tile([128, nk, 128], F16, tag="vt",
                                         bufs=8, name=f"vt_{s}{b}{h}")
                            nc.sync.dma_start(
                                vt[:],
                                sc[f"v_{s}"][b * c.S:(b + 1) * c.S,
                                             h * 128:(h + 1) * 128]
                                .rearrange("(n p) j -> p n j", p=128))
                            VT[(s, b, h)] = vt

                for s in ("x", "y"):
                    # one shared o buffer; stream y reuses x's after wo-x
                    o_sb = p2o.tile([128, c.NQT, c.T], F16, tag="osb",
                                    bufs=1, name="o_sb")

                    for b in range(c.B):
                        for h in range(c.NQT):
                            vt = VT[(s, b, h)]
                            for q0 in range(0, c.S, c.QCH):
                                qsl = slice(b * c.S + q0,
                                            b * c.S + q0 + c.QCH)
                                o_ps = mm([128, c.QCH], "o_ps")
                                e_acc = p2.tile([128, c.QCH], F16,
                                                tag="eacc", bufs=2,
                                                name="eacc")
                                for ik in range(nk):
                                    ksl = slice(b * c.S + ik * 128,
                                                b * c.S + (ik + 1) * 128)
                                    s_ps = mm([128, c.QCH], "s_ps")
                                    nc.tensor.matmul(
                                        s_ps[:], KT[s][:, h, ksl],
                                        QT[s][:, h, qsl],
                                        start=True, stop=True)
                                    e16 = p2.tile([128, c.QCH], F16,
                                                  tag="e16", bufs=4,
                                                  name="e16")
                                    nc.scalar.activation(e16[:], s_ps[:],
                                                         AF.Exp)
                                    if ik == 0:
                                        nc.vector.tensor_copy(e_acc[:],
                                                              e16[:])
                                    else:
                                        nc.vector.tensor_add(e_acc[:],
                                                             e_acc[:],
                                                             e16[:])
                                    nc.tensor.matmul(
                                        o_ps[:], vt[:, ik], e16[:],
                                        start=(ik == 0),
                                        stop=(ik == nk - 1))
                                sum_ps = row([1, c.QCH], "sum_ps")
                                nc.tensor.matmul(sum_ps[:], ones_col[:],
                                                 e_acc[:],
                                                 start=True, stop=True)
                                rs_ = p2.tile([1, c.QCH], F32, tag="rs",
                                              name="rs")
                                nc.vector.reciprocal(rs_[:], sum_ps[:])
                                rs16 = p2.tile([1, c.QCH], F16,
                                               tag="rs16", name="rs16")
                                nc.vector.tensor_copy(rs16[:], rs_[:])
                                bc16 = bcast_free(rs16, c.QCH, p2, "at")
                                nc.vector.tensor_mul(o_sb[:, h, qsl],
                                                     o_ps[:], bc16[:])

                    # ---- wo partial for ALL tokens, window-major ----
                    for w in range(NCORES):
                        for u in range(c.TC // c.TCW):
                            t0 = w * c.TC + u * c.TCW
                            for dt in range(c.DT):
                                hp = mm([128, c.TCW], "hp")
                                for o in range(c.NQT):
                                    nc.tensor.matmul(
                                        hp[:],
                                        WO[s][:, o, dt * 128:(dt + 1) * 128],
                                        o_sb[:, o, t0:t0 + c.TCW],
                                        start=(o == 0),
                                        stop=(o == c.NQT - 1))
                                hp16 = p2w.tile([128, c.TCW], F16,
                                                tag="hp16", bufs=6,
                                                name="hp16")
                                if dt % 2 == 0:
                                    nc.vector.tensor_copy(hp16[:], hp[:])
                                else:
                                    nc.scalar.activation(hp16[:], hp[:],
                                                         AF.Copy)
                                nc.sync.dma_start(
                                    sc[f"hp_{s}"][w * c.D + dt * 128:
                                                  w * c.D + (dt + 1) * 128,
                                                  u * c.TCW:
                                                  (u + 1) * c.TCW],
                                    hp16[:])

                    nc.gpsimd.collective_compute(
                        "ReduceScatter", mybir.AluOpType.add,
                        replica_groups=[list(range(NCORES))],
                        ins=[sc[f"hp_{s}"][:].opt()],
                        outs=[sc[f"h_{s}"][:].opt()],
                    )
                    prefetch_p3(s)

          # ======== PHASE 3: SwiGLU FFN + residual + post-norm ========
          # FF is processed in two half-passes (NFH tiles each) so the
          # zg/w2 tiles stay small enough to overlap with the prefetch
          # pool; r_all accumulates across the passes.
          with (
            tc.tile_pool(name="p3", bufs=1) as p3,
            tc.tile_pool(name="p3w", bufs=2) as p3w,
            tc.tile_pool(name="p3s", bufs=2) as p3s,
          ):
            for s in ("x", "y"):
                fnorm = pf[s]["fnorm"]
                for icw in range(c.TC // c.TCW):
                    tw = c.TCW
                    wsl = slice(icw * tw, (icw + 1) * tw)
                    # h in 4 chunk-tiles so the first z matmul starts
                    # after ~0.5 MB of DMA instead of 2 MB
                    HCH = []
                    for hc in range(4):
                        ht = p3.tile([128, c.DT // 4, tw], F16, tag="h4",
                                     bufs=8, name=f"h4_{hc}")
                        nc.scalar.dma_start(
                            ht[:],
                            sc[f"h_{s}"][hc * (c.D // 4):
                                         (hc + 1) * (c.D // 4), wsl]
                            .rearrange("(o p) t -> p o t", p=128))
                        HCH.append(ht)

                    def hsl(o):
                        return HCH[o // 4][:, o % 4]

                    r_all = p3.tile([128, c.DT, tw], F32, tag="r",
                                    name="r_all")
                    nacc = p3.tile([128, tw], F16, tag="nacc",
                                   name="nacc")
                    for half in range(2):
                        zg = p3.tile([128, NFH, tw], F16, tag="zg",
                                     name="zg")
                        f0 = half * NFH
                        assert NFH % 2 == 0
                        for fb in range(NFH // 2):
                            if s == "x" and icw == 0 and half == 0 \
                                    and fb == 0:
                                w1, w3 = pf[s]["w1"], pf[s]["w3"]
                            else:
                                fsl = slice((f0 + fb * 2) * 128,
                                            (f0 + fb * 2 + 2) * 128)
                                w1 = p3w.tile([128, c.DT, 256], F16,
                                              tag="w1", name="w1")
                                nc.scalar.dma_start(
                                    w1[:], ins[f"w1T_{s}"][:, fsl]
                                    .rearrange("(o p) j -> p o j", p=128))
                                w3 = p3w.tile([128, c.DT, 256], F16,
                                              tag="w3", name="w3")
                                nc.scalar.dma_start(
                                    w3[:], ins[f"w3T_{s}"][:, fsl]
                                    .rearrange("(o p) j -> p o j", p=128))
                            for sub in range(2):
                                ftl = fb * 2 + sub
                                jsl = slice(sub * 128, (sub + 1) * 128)
                                z1 = mm([128, tw], "z1")
                                z3 = mm([128, tw], "z3")
                                for o in range(c.DT):
                                    nc.tensor.matmul(z1[:], w1[:, o, jsl],
                                                     hsl(o),
                                                     start=(o == 0),
                                                     stop=(o == c.DT - 1))
                                for o in range(c.DT):
                                    nc.tensor.matmul(z3[:], w3[:, o, jsl],
                                                     hsl(o),
                                                     start=(o == 0),
                                                     stop=(o == c.DT - 1))
                                sg = p3s.tile([128, tw], F16, tag="sg",
                                              name="sg")
                                nc.scalar.activation(sg[:], z1[:],
                                                     AF.Sigmoid)
                                sl = p3s.tile([128, tw], F16, tag="sl",
                                              name="sl")
                                nc.vector.tensor_mul(sl[:], z1[:], sg[:])
                                nc.vector.tensor_mul(zg[:, ftl], z3[:],
                                                     sl[:])

                        assert c.DT % 2 == 0
                        for db in range(c.DT // 2):
                            w2 = p3w.tile([128, NFH, 256], F16, tag="w2",
                                          name="w2")
                            nc.scalar.dma_start(
                                w2[:],
                                ins[f"w2T_{s}"][f0 * 128:
                                                (f0 + NFH) * 128,
                                                db * 256:(db + 1) * 256]
                                .rearrange("(o p) j -> p o j", p=128))
                            for sub in range(2):
                                dt = db * 2 + sub
                                jsl = slice(sub * 128, (sub + 1) * 128)
                                fp = mm([128, tw], "fp")
                                for ftl in range(NFH):
                                    nc.tensor.matmul(fp[:],
                                                     w2[:, ftl, jsl],
                                                     zg[:, ftl],
                                                     start=(ftl == 0),
                                                     stop=(ftl == NFH - 1))
                                if half == 0:
                                    res = p3s.tile([128, tw], F16,
                                                   tag="res", bufs=2,
                                                   name="res")
                                    nc.scalar.dma_start(
                                        res[:],
                                        ins[f"res_{s}"][dt * 128:
                                                        (dt + 1) * 128,
                                                        wsl])
                                    nc.vector.tensor_add(r_all[:, dt],
                                                         fp[:], res[:])
                                else:
                                    nc.vector.tensor_add(r_all[:, dt],
                                                         r_all[:, dt],
                                                         fp[:])
                                    r2 = p3s.tile([128, tw], F16,
                                                  tag="r2", name="r2")
                                    nc.vector.tensor_mul(r2[:],
                                                         r_all[:, dt],
                                                         r_all[:, dt])
                                    if dt == 0:
                                        nc.vector.tensor_copy(nacc[:],
                                                              r2[:])
                                    else:
                                        nc.vector.tensor_add(nacc[:],
                                                             nacc[:],
                                                             r2[:])
                    ns_ps = row([1, tw], "ns")
                    nc.tensor.matmul(ns_ps[:], ones_col[:], nacc[:],
                                     start=True, stop=True)
                    rmsn = p3s.tile([1, tw], F32, tag="rmsn", name="rmsn")
                    nc.scalar.activation(rmsn[:], ns_ps[:], AF.Sqrt,
                                         bias=eps1[:], scale=one_over_d)
                    rsqn = p3s.tile([1, tw], F32, tag="rsqn", name="rsqn")
                    nc.vector.reciprocal(rsqn[:], rmsn[:])
                    rsqn16 = p3s.tile([1, tw], F16, tag="rsqn16",
                                      name="rsqn16")
                    nc.vector.tensor_copy(rsqn16[:], rsqn[:])
                    bcn = bcast_free(rsqn16, tw, p3s, f"fn{s}")
                    for dt in range(c.DT):
                        nc.vector.tensor_mul(r_all[:, dt], r_all[:, dt],
                                             bcn[:])
                        ofn = p3s.tile([128, tw], F32, tag="ofn",
                                       name="ofn")
                        nc.scalar.activation(ofn[:], r_all[:, dt], AF.Copy,
                                             scale=fnorm[:, dt:dt + 1])
                        nc.sync.dma_start(
                            outs[s][dt * 128:(dt + 1) * 128, wsl], ofn[:])


# ======================= host-side wrapper =========================

_CACHE = {}


def _prep_inputs(cfg, x, y, attn_norm_w,
                 wq_x, wk_x, wv_x, wo_x, wq_y, wk_y, wv_y, wo_y,
                 w1_x, w2_x, w3_x, ffn_norm_x,
                 w1_y, w2_y, w3_y, ffn_norm_y):
    c = cfg
    f16 = np.float16
    nw = np.asarray(attn_norm_w, np.float32)
    qscale = nw / np.sqrt(c.HD)

    def t16(a):
        return np.ascontiguousarray(np.asarray(a, np.float32).T).astype(f16)

    per_core = [dict() for _ in range(NCORES)]
    shared = {}
    for s, (xv, wq, wk, wv, wo, w1, w2, w3, fn) in {
        "x": (x, wq_x, wk_x, wv_x, wo_x, w1_x, w2_x, w3_x, ffn_norm_x),
        "y": (y, wq_y, wk_y, wv_y, wo_y, w1_y, w2_y, w3_y, ffn_norm_y),
    }.items():
        xt = np.asarray(xv, np.float32).reshape(c.T, c.D).T  # [D, T]
        xt16 = np.ascontiguousarray(xt).astype(f16)
        shared[f"{s}T"] = xt16
        wqT = (np.asarray(wq, np.float32) * qscale[None, :]).T  # [D, D]
        wkT = (np.asarray(wk, np.float32) * nw[None, :]).T
        wvT = (np.asarray(wv, np.float32) * nw[None, :]).T
        woT = np.asarray(wo, np.float32).T                     # [Din, Dout]
        shared[f"w1T_{s}"] = t16(w1)
        shared[f"w3T_{s}"] = t16(w3)
        shared[f"w2T_{s}"] = t16(w2)
        shared[f"fnorm_{s}"] = np.ascontiguousarray(
            np.asarray(fn, np.float32).reshape(c.DT, 128).T)
        for r in range(NCORES):
            js = slice(r * c.NQ, (r + 1) * c.NQ)
            ts = slice(r * c.TC, (r + 1) * c.TC)
            per_core[r][f"wqT_{s}"] = np.ascontiguousarray(wqT[:, js]).astype(f16)
            per_core[r][f"wkT_{s}"] = np.ascontiguousarray(wkT[:, js]).astype(f16)
            per_core[r][f"wvT_{s}"] = np.ascontiguousarray(wvT[:, js]).astype(f16)
            per_core[r][f"woT_{s}"] = np.ascontiguousarray(woT[js, :]).astype(f16)
            per_core[r][f"res_{s}"] = np.ascontiguousarray(xt16[:, ts])
    in_maps = []
    for r in range(NCORES):
        m = dict(shared)
        m.update(per_core[r])
        in_maps.append(m)
    return in_maps


def run(cfg, inputs, **kw):
    from concourse import bass_utils

    key = (cfg.B, cfg.S, cfg.D, cfg.H, cfg.HD, cfg.FF)
    if key not in _CACHE:
        _CACHE[key] = build(cfg)
    nc = _CACHE[key]
    in_maps = _prep_inputs(cfg, **{k: v for k, v in inputs.items()
                                   if k != "start_pos"})
    res = bass_utils.run_bass_kernel_spmd(
        nc, in_maps, core_ids=list(range(NCORES)), **kw)
    outs = []
    for s in ("x", "y"):
        cols = [res.results[r][f"out_{s}"] for r in range(NCORES)]
        full_t = np.concatenate(cols, axis=1)           # [D, T]
        outs.append(np.ascontiguousarray(full_t.T)
                    .reshape(cfg.B, cfg.S, cfg.D).astype(np.float32))
    return tuple(outs), res


def kernel(**inputs):
    (out_x, out_y), _ = run(FULL, inputs)
    return out_x, out_y


# revision 13
# speedup vs baseline: 1.0863x; 1.0863x over previous
"""CrossAttentionBlockLLaMA on 8 Trainium2 NeuronCores (Bass/Tile).

Sharding:
  - QKV + attention: tensor-parallel over heads (2 heads/core).
  - Output projection wo: row-sharded over heads; each core computes a
    partial h for ALL tokens, written window-major [8, D, TC]; a
    ReduceScatter sums partials and hands core r exactly h.T[:, tokens_r].
  - FFN + post-norm: token-parallel (TC tokens/core), full weights.

Perf structure (v2):
  - q/k/v and attention output o live entirely in SBUF (no DRAM
    round-trip), so attention + wo need no input DMA and keep the PE
    fed while the ReduceScatter of the other stream is on the wire.
  - Row-sums (softmax denominators, RMS mean-squares) accumulate on the
    Vector engine via f16 tile adds; only ONE ones-matmul per reduction
    hits the PE instead of 16.
  - Residual is read f16 straight from the pre-transposed activations.

Layouts: host pre-transposes activations/weights so every matmul's
contraction dim is on SBUF partitions. attn_norm_w and 1/sqrt(HD) are
folded into wq/wk/wv host-side; per-token 1/rms factors are applied to
q/k/v on device. Matmul inputs fp16, PSUM accumulation fp32.

Self-contained: hardcodes shapes from the problem spec.
"""
import numpy as np

NCORES = 8
EPS = 1e-5


class Cfg:
    def __init__(self, B=2, S=2048, D=2048, H=16, HD=128, FF=5632):
        self.B, self.S, self.D, self.H, self.HD, self.FF = B, S, D, H, HD, FF
        self.T = B * S                    # total tokens
        self.TC = self.T // NCORES        # tokens per core (phase 3)
        self.NQ = (H // NCORES) * HD      # per-core head dims
        self.DT = D // 128                # d-tiles
        self.FT = FF // 128               # ff-tiles
        self.NQT = self.NQ // 128         # per-core head-dim tiles
        self.TCH = min(512, self.T)       # phase-1 token chunk
        self.QCH = min(512, S)            # phase-2 query chunk
        self.TCW = min(512, self.TC)      # phase-3 / wo token chunk
        assert self.T % self.TCH == 0 and S % self.QCH == 0
        assert self.TC % self.TCW == 0 and S % 128 == 0
        assert HD == 128 and D % 128 == 0 and FF % 128 == 0


FULL = Cfg()


def build(cfg=FULL):
    import concourse.mybir as mybir
    import concourse.tile as tile
    from concourse import bacc

    F16 = mybir.dt.float16
    F32 = mybir.dt.float32

    c = cfg
    nc = bacc.Bacc("TRN2", target_bir_lowering=False, debug=False,
                   num_devices=NCORES)

    ins = {}
    outs = {}
    for s in ("x", "y"):
        ins[f"{s}T"] = nc.dram_tensor(f"{s}T", [c.D, c.T], F16,
                                      kind="ExternalInput").ap()
        for w in ("wq", "wk", "wv"):
            ins[f"{w}T_{s}"] = nc.dram_tensor(
                f"{w}T_{s}", [c.D, c.NQ], F16, kind="ExternalInput").ap()
        ins[f"woT_{s}"] = nc.dram_tensor(
            f"woT_{s}", [c.NQ, c.D], F16, kind="ExternalInput").ap()
        ins[f"w1P_{s}"] = nc.dram_tensor(
            f"w1P_{s}", [128, c.FF, c.DT], F16, kind="ExternalInput").ap()
        ins[f"w3P_{s}"] = nc.dram_tensor(
            f"w3P_{s}", [128, c.FF, c.DT], F16, kind="ExternalInput").ap()
        ins[f"w2P_{s}"] = nc.dram_tensor(
            f"w2P_{s}", [128, 2, c.D, c.FT // 2], F16,
            kind="ExternalInput").ap()
        ins[f"res_{s}"] = nc.dram_tensor(
            f"res_{s}", [128, c.DT, c.TC], F16, kind="ExternalInput").ap()
        ins[f"fnorm_{s}"] = nc.dram_tensor(
            f"fnorm_{s}", [128, c.DT], F32, kind="ExternalInput").ap()
        outs[s] = nc.dram_tensor(f"out_{s}", [c.D, c.TC], F32,
                                 kind="ExternalOutput").ap()

    with tile.TileContext(nc) as tc:
        _emit(tc, nc, c, ins, outs)
    nc.compile()
    return nc


def _emit(tc, nc, c, ins, outs):
    import concourse.mybir as mybir

    F16 = mybir.dt.float16
    F32 = mybir.dt.float32
    AF = mybir.ActivationFunctionType
    one_over_d = 1.0 / c.D

    with (
        tc.tile_pool(name="psum", bufs=1, space="PSUM") as ps,
        tc.tile_pool(name="const", bufs=1) as const,
        tc.tile_pool(name="dram", bufs=1, space="DRAM") as dram,
    ):
        ones_col = const.tile([128, 1], F16)
        nc.vector.memset(ones_col[:], 1.0)
        eps1 = const.tile([1, 1], F32)
        nc.vector.memset(eps1[:], EPS)

        sc = {}
        for s in ("x", "y"):
            # v in partition-major per-head layout: contiguous 4KB loads
            sc[f"v_{s}"] = dram.tile([128, c.NQT, c.T // 128, 128], F16,
                                     name=f"v_{s}")
            # wo partials, window-major, partition-major rows: core w's
            # shard is rows [w*128,(w+1)*128) = a contiguous 2 MB block
            sc[f"hp_{s}"] = dram.tile([NCORES * 128, c.DT * c.TC], F16,
                                      name=f"hp_{s}")
            sc[f"h_{s}"] = dram.tile([128, c.DT, c.TC], F16,
                                     name=f"h_{s}")

        def mm(shape, name):
            return ps.tile(shape, F32, tag="mm", bufs=6, name=name)

        def row(shape, name):
            return ps.tile(shape, F32, tag="row", bufs=2, name=name)

        def bcast_free(rsq16, width, sb_pool, name):
            """[1,width] f16 -> [128,width] f16 on the (idle) GpSimd
            engine: no DMA queue traffic, no DRAM round-trip."""
            bc16 = sb_pool.tile([128, width], F16, tag="bc16",
                                name=f"bc16_{name}")
            nc.gpsimd.partition_broadcast(bc16[:], rsq16[:1, :width])
            return bc16

        NFH = c.FT // 2  # ff tiles per phase-3 half-pass

        # small long-lived pool: cross-phase prefetch targets
        with tc.tile_pool(name="pfp", bufs=1) as pfp:
          pf = {}

          def prefetch_p3(s):
            """First-chunk loads for phase 3, issued from the gpsimd queue
            right after the ReduceScatter trigger so they land while the
            PE still works on phase-2 compute."""
            fnorm = pfp.tile([128, c.DT], F32, tag="fnorm", bufs=2,
                             name=f"fnorm_{s}")
            nc.gpsimd.dma_start(fnorm[:], ins[f"fnorm_{s}"])
            pf[s] = {"fnorm": fnorm}
            if s != "x":
                return
            w1 = pfp.tile([128, 256, c.DT], F16, tag="w1c0", name="w1c0")
            nc.gpsimd.dma_start(w1[:], ins[f"w1P_{s}"][:, :256, :])
            w3 = pfp.tile([128, 256, c.DT], F16, tag="w3c0", name="w3c0")
            nc.gpsimd.dma_start(w3[:], ins[f"w3P_{s}"][:, :256, :])
            pf[s].update(w1=w1, w3=w3)

          with tc.tile_pool(name="qko", bufs=1) as qko:
            # persistent SBUF q/k per stream (8 MB total); v spills to DRAM
            QT, KT = {}, {}
            for s in ("x", "y"):
                QT[s] = qko.tile([128, c.NQT, c.T], F16, name=f"qT_{s}")
                KT[s] = qko.tile([128, c.NQT, c.T], F16, name=f"kT_{s}")

            # ======== PHASE 1: RMSNorm stats + QKV projections ========
            with (
                tc.tile_pool(name="p1w", bufs=1) as p1w,
                tc.tile_pool(name="p1a", bufs=2) as p1a,
                tc.tile_pool(name="p1s", bufs=2) as p1s,
            ):
                W = {}
                for s in ("x", "y"):
                    for w in ("wq", "wk", "wv"):
                        t = p1w.tile([128, c.DT, c.NQ], F16,
                                     name=f"{w}_{s}_sb")
                        nc.gpsimd.dma_start(
                            t[:],
                            ins[f"{w}T_{s}"].rearrange("(o p) j -> p o j",
                                                       p=128))
                        W[f"{w}{s}"] = t

                for ich in range(c.T // c.TCH):
                    tsl = slice(ich * c.TCH, (ich + 1) * c.TCH)
                    act = {}
                    rsq_free = {}
                    rsq_part = {}
                    for s in ("x", "y"):
                        at = p1a.tile([128, c.DT, c.TCH], F16,
                                      tag=f"act_{s}", name=f"act_{s}")
                        nc.sync.dma_start(
                            at[:],
                            ins[f"{s}T"][:, tsl].rearrange(
                                "(o p) t -> p o t", p=128))
                        act[s] = at

                        # mean-square accumulate on DVE, one PE matmul
                        acc = p1s.tile([128, c.TCH], F16, tag="msacc",
                                       bufs=2, name=f"msacc_{s}")
                        nc.vector.tensor_mul(acc[:], at[:, 0], at[:, 0])
                        for o in range(1, c.DT):
                            sq = p1s.tile([128, c.TCH], F16, tag="sq",
                                          name=f"sq_{s}{o}")
                            nc.vector.tensor_mul(sq[:], at[:, o], at[:, o])
                            nc.vector.tensor_add(acc[:], acc[:], sq[:])
                        ms_ps = row([1, c.TCH], f"ms_{s}")
                        nc.tensor.matmul(ms_ps[:], ones_col[:], acc[:],
                                         start=True, stop=True)
                        rms = p1s.tile([1, c.TCH], F32, tag="rms",
                                       bufs=1, name=f"rms_{s}")
                        nc.scalar.activation(rms[:], ms_ps[:], AF.Sqrt,
                                             bias=eps1[:], scale=one_over_d)
                        rsqf = p1s.tile([1, c.TCH], F32, tag="rsqf",
                                        bufs=1, name=f"rsqf_{s}")
                        nc.vector.reciprocal(rsqf[:], rms[:])
                        rsqf16 = p1s.tile([1, c.TCH], F16, tag="rsqf16",
                                          name=f"rsqf16_{s}")
                        nc.vector.tensor_copy(rsqf16[:], rsqf[:])
                        rsq_free[s] = rsqf16

                        nsub = c.TCH // 128
                        rfd = dram.tile([1, c.TCH], F32, tag="rsq_row",
                                        bufs=4, name=f"rfd_{s}")
                        nc.scalar.dma_start(rfd[:], rsqf[:])
                        rsqT = p1s.tile([128, nsub], F32, tag="rsqT",
                                        name=f"rsqT_{s}")
                        nc.scalar.dma_start(
                            rsqT[:],
                            rfd[0, :].rearrange("(n p) -> p n", p=128))
                        rsq_part[s] = rsqT

                    for s in ("x", "y"):
                        kv = "y" if s == "x" else "x"
                        bc_q = bcast_free(rsq_free[s], c.TCH, p1s,
                                          f"q{s}{ich}")
                        bc_k = bcast_free(rsq_free[kv], c.TCH, p1s,
                                          f"k{s}{ich}")

                        for (wname, src, bc, dst) in (
                            ("wq", s, bc_q, QT[s]),
                            ("wk", kv, bc_k, KT[s]),
                        ):
                            for jt in range(c.NQT):
                                pm = mm([128, c.TCH], f"{wname}{s}{jt}")
                                wt = W[f"{wname}{s}"]
                                for o in range(c.DT):
                                    nc.tensor.matmul(
                                        pm[:],
                                        wt[:, o, jt * 128:(jt + 1) * 128],
                                        act[src][:, o],
                                        start=(o == 0),
                                        stop=(o == c.DT - 1))
                                nc.vector.tensor_mul(dst[:, jt, tsl],
                                                     pm[:], bc[:])

                        for i in range(c.TCH // 128):
                            pv = mm([128, c.NQ], f"v{s}{i}")
                            for o in range(c.DT):
                                nc.tensor.matmul(
                                    pv[:],
                                    act[kv][:, o, i * 128:(i + 1) * 128],
                                    W[f"wv{s}"][:, o, :],
                                    start=(o == 0), stop=(o == c.DT - 1))
                            vt = p1s.tile([128, c.NQ], F16, tag="v_out",
                                          name=f"v{s}{i}o")
                            nc.vector.tensor_scalar_mul(
                                vt[:], pv[:], rsq_part[kv][:, i:i + 1])
                            irow = ich * (c.TCH // 128) + i
                            for hh in range(c.NQT):
                                nc.sync.dma_start(
                                    sc[f"v_{s}"][:, hh, irow, :],
                                    vt[:, hh * 128:(hh + 1) * 128])

            # ======== PHASE 2: attention + wo partial + ReduceScatter ====
            with (
                tc.tile_pool(name="p2", bufs=2) as p2,
                tc.tile_pool(name="p2o", bufs=1) as p2o,
                tc.tile_pool(name="p2w", bufs=2) as p2w,
            ):
                nk = c.S // 128
                # hoist ALL attention/wo input loads: nothing below needs
                # the sync DMA queue once the hp stores start flowing
                VT, WO = {}, {}
                for s in ("x", "y"):
                    WO[s] = p2w.tile([128, c.NQT, c.D], F16, tag="wo",
                                     bufs=2, name=f"wo_{s}")
                    nc.sync.dma_start(
                        WO[s][:],
                        ins[f"woT_{s}"].rearrange("(o p) j -> p o j",
                                                  p=128))
                for s in ("x", "y"):
                    for b in range(c.B):
                        for h in range(c.NQT):
                            vt = p2.tile([128, nk, 128], F16, tag="vt",
                                         bufs=8, name=f"vt_{s}{b}{h}")
                            nc.sync.dma_start(
                                vt[:],
                                sc[f"v_{s}"][:, h,
                                             b * nk:(b + 1) * nk, :])
                            VT[(s, b, h)] = vt

                for s in ("x", "y"):
                    # one shared o buffer; stream y reuses x's after wo-x
                    o_sb = p2o.tile([128, c.NQT, c.T], F16, tag="osb",
                                    bufs=1, name="o_sb")

                    for b in range(c.B):
                        for h in range(c.NQT):
                            vt = VT[(s, b, h)]
                            for q0 in range(0, c.S, c.QCH):
                                qsl = slice(b * c.S + q0,
                                            b * c.S + q0 + c.QCH)
                                o_ps = mm([128, c.QCH], "o_ps")
                                e_acc = p2.tile([128, c.QCH], F16,
                                                tag="eacc", bufs=2,
                                                name="eacc")
                                for ik in range(nk):
                                    ksl = slice(b * c.S + ik * 128,
                                                b * c.S + (ik + 1) * 128)
                                    s_ps = mm([128, c.QCH], "s_ps")
                                    nc.tensor.matmul(
                                        s_ps[:], KT[s][:, h, ksl],
                                        QT[s][:, h, qsl],
                                        start=True, stop=True)
                                    e16 = p2.tile([128, c.QCH], F16,
                                                  tag="e16", bufs=4,
                                                  name="e16")
                                    nc.scalar.activation(e16[:], s_ps[:],
                                                         AF.Exp)
                                    if ik == 0:
                                        nc.vector.tensor_copy(e_acc[:],
                                                              e16[:])
                                    else:
                                        nc.vector.tensor_add(e_acc[:],
                                                             e_acc[:],
                                                             e16[:])
                                    nc.tensor.matmul(
                                        o_ps[:], vt[:, ik], e16[:],
                                        start=(ik == 0),
                                        stop=(ik == nk - 1))
                                sum_ps = row([1, c.QCH], "sum_ps")
                                nc.tensor.matmul(sum_ps[:], ones_col[:],
                                                 e_acc[:],
                                                 start=True, stop=True)
                                rs_ = p2.tile([1, c.QCH], F32, tag="rs",
                                              name="rs")
                                nc.vector.reciprocal(rs_[:], sum_ps[:])
                                rs16 = p2.tile([1, c.QCH], F16,
                                               tag="rs16", name="rs16")
                                nc.vector.tensor_copy(rs16[:], rs_[:])
                                bc16 = bcast_free(rs16, c.QCH, p2, "at")
                                nc.vector.tensor_mul(o_sb[:, h, qsl],
                                                     o_ps[:], bc16[:])

                    # ---- wo partial for ALL tokens, window-major ----
                    for w in range(NCORES):
                        for u in range(c.TC // c.TCW):
                            t0 = w * c.TC + u * c.TCW
                            for dt in range(c.DT):
                                hp = mm([128, c.TCW], "hp")
                                for o in range(c.NQT):
                                    nc.tensor.matmul(
                                        hp[:],
                                        WO[s][:, o, dt * 128:(dt + 1) * 128],
                                        o_sb[:, o, t0:t0 + c.TCW],
                                        start=(o == 0),
                                        stop=(o == c.NQT - 1))
                                hp16 = p2w.tile([128, c.TCW], F16,
                                                tag="hp16", bufs=6,
                                                name="hp16")
                                if dt % 2 == 0:
                                    nc.vector.tensor_copy(hp16[:], hp[:])
                                else:
                                    nc.scalar.activation(hp16[:], hp[:],
                                                         AF.Copy)
                                nc.sync.dma_start(
                                    sc[f"hp_{s}"][w * 128:(w + 1) * 128,
                                                  dt * c.TC + u * c.TCW:
                                                  dt * c.TC +
                                                  (u + 1) * c.TCW],
                                    hp16[:])

                    nc.gpsimd.collective_compute(
                        "ReduceScatter", mybir.AluOpType.add,
                        replica_groups=[list(range(NCORES))],
                        ins=[sc[f"hp_{s}"][:].opt()],
                        outs=[sc[f"h_{s}"][:].opt()],
                    )
                    prefetch_p3(s)

          # ======== PHASE 3: SwiGLU FFN + residual + post-norm ========
          # FF is processed in two half-passes (NFH tiles each) so the
          # zg/w2 tiles stay small enough to overlap with the prefetch
          # pool; r_all accumulates across the passes.
          with (
            tc.tile_pool(name="p3", bufs=1) as p3,
            tc.tile_pool(name="p3w", bufs=2) as p3w,
            tc.tile_pool(name="p3s", bufs=2) as p3s,
          ):
            for s in ("x", "y"):
                fnorm = pf[s]["fnorm"]
                for icw in range(c.TC // c.TCW):
                    tw = c.TCW
                    wsl = slice(icw * tw, (icw + 1) * tw)
                    # h in 4 chunk-tiles, contiguous loads on the idle
                    # gpsimd queue (runs in the post-RS quiet window)
                    HCH = []
                    for hc in range(4):
                        ht = p3.tile([128, c.DT // 4, tw], F16, tag="h4",
                                     bufs=8, name=f"h4_{hc}")
                        nc.gpsimd.dma_start(
                            ht[:], sc[f"h_{s}"][:, hc * 4:(hc + 1) * 4,
                                                wsl])
                        HCH.append(ht)

                    def hsl(o):
                        return HCH[o // 4][:, o % 4]

                    res_all = p3.tile([128, c.DT, tw], F16, tag="resa",
                                      name="res_all")
                    nc.gpsimd.dma_start(res_all[:],
                                        ins[f"res_{s}"][:, :, wsl])

                    r_all = p3.tile([128, c.DT, tw], F32, tag="r",
                                    name="r_all")
                    nacc = p3.tile([128, tw], F16, tag="nacc",
                                   name="nacc")
                    for half in range(2):
                        zg = p3.tile([128, NFH, tw], F16, tag="zg",
                                     name="zg")
                        f0 = half * NFH
                        assert NFH % 2 == 0
                        for fb in range(NFH // 2):
                            if s == "x" and icw == 0 and half == 0 \
                                    and fb == 0:
                                w1, w3 = pf[s]["w1"], pf[s]["w3"]
                            else:
                                fsl = slice((f0 + fb * 2) * 128,
                                            (f0 + fb * 2 + 2) * 128)
                                w1 = p3w.tile([128, 256, c.DT], F16,
                                              tag="w1", name="w1")
                                nc.scalar.dma_start(
                                    w1[:], ins[f"w1P_{s}"][:, fsl, :])
                                w3 = p3w.tile([128, 256, c.DT], F16,
                                              tag="w3", name="w3")
                                nc.scalar.dma_start(
                                    w3[:], ins[f"w3P_{s}"][:, fsl, :])
                            for sub in range(2):
                                ftl = fb * 2 + sub
                                jsl = slice(sub * 128, (sub + 1) * 128)
                                z1 = mm([128, tw], "z1")
                                z3 = mm([128, tw], "z3")
                                for o in range(c.DT):
                                    nc.tensor.matmul(z1[:], w1[:, jsl, o],
                                                     hsl(o),
                                                     start=(o == 0),
                                                     stop=(o == c.DT - 1))
                                for o in range(c.DT):
                                    nc.tensor.matmul(z3[:], w3[:, jsl, o],
                                                     hsl(o),
                                                     start=(o == 0),
                                                     stop=(o == c.DT - 1))
                                sg = p3s.tile([128, tw], F16, tag="sg",
                                              name="sg")
                                nc.scalar.activation(sg[:], z1[:],
                                                     AF.Sigmoid)
                                sl = p3s.tile([128, tw], F16, tag="sl",
                                              name="sl")
                                nc.vector.tensor_mul(sl[:], z1[:], sg[:])
                                nc.vector.tensor_mul(zg[:, ftl], z3[:],
                                                     sl[:])

                        assert c.DT % 2 == 0
                        for db in range(c.DT // 2):
                            w2 = p3w.tile([128, 256, NFH], F16, tag="w2",
                                          name="w2")
                            nc.gpsimd.dma_start(
                                w2[:],
                                ins[f"w2P_{s}"][:, half,
                                                db * 256:(db + 1) * 256,
                                                :])
                            for sub in range(2):
                                dt = db * 2 + sub
                                jsl = slice(sub * 128, (sub + 1) * 128)
                                fp = mm([128, tw], "fp")
                                for ftl in range(NFH):
                                    nc.tensor.matmul(fp[:],
                                                     w2[:, jsl, ftl],
                                                     zg[:, ftl],
                                                     start=(ftl == 0),
                                                     stop=(ftl == NFH - 1))
                                if half == 0:
                                    nc.vector.tensor_add(r_all[:, dt],
                                                         fp[:],
                                                         res_all[:, dt])
                                else:
                                    nc.vector.tensor_add(r_all[:, dt],
                                                         r_all[:, dt],
                                                         fp[:])
                                    r2 = p3s.tile([128, tw], F16,
                                                  tag="r2", name="r2")
                                    nc.vector.tensor_mul(r2[:],
                                                         r_all[:, dt],
                                                         r_all[:, dt])
                                    if dt == 0:
                                        nc.vector.tensor_copy(nacc[:],
                                                              r2[:])
                                    else:
                                        nc.vector.tensor_add(nacc[:],
                                                             nacc[:],
                                                             r2[:])
                    ns_ps = row([1, tw], "ns")
                    nc.tensor.matmul(ns_ps[:], ones_col[:], nacc[:],
                                     start=True, stop=True)
                    rmsn = p3s.tile([1, tw], F32, tag="rmsn", name="rmsn")
                    nc.scalar.activation(rmsn[:], ns_ps[:], AF.Sqrt,
                                         bias=eps1[:], scale=one_over_d)
                    rsqn = p3s.tile([1, tw], F32, tag="rsqn", name="rsqn")
                    nc.vector.reciprocal(rsqn[:], rmsn[:])
                    rsqn16 = p3s.tile([1, tw], F16, tag="rsqn16",
                                      name="rsqn16")
                    nc.vector.tensor_copy(rsqn16[:], rsqn[:])
                    bcn = bcast_free(rsqn16, tw, p3s, f"fn{s}")
                    for dt in range(c.DT):
                        nc.vector.tensor_mul(r_all[:, dt], r_all[:, dt],
                                             bcn[:])
                        ofn = p3s.tile([128, tw], F32, tag="ofn",
                                       name="ofn")
                        nc.scalar.activation(ofn[:], r_all[:, dt], AF.Copy,
                                             scale=fnorm[:, dt:dt + 1])
                        nc.sync.dma_start(
                            outs[s][dt * 128:(dt + 1) * 128, wsl], ofn[:])


# ======================= host-side wrapper =========================

_CACHE = {}


def _prep_inputs(cfg, x, y, attn_norm_w,
                 wq_x, wk_x, wv_x, wo_x, wq_y, wk_y, wv_y, wo_y,
                 w1_x, w2_x, w3_x, ffn_norm_x,
                 w1_y, w2_y, w3_y, ffn_norm_y):
    c = cfg
    f16 = np.float16
    nw = np.asarray(attn_norm_w, np.float32)
    qscale = nw / np.sqrt(c.HD)

    def t16(a):
        return np.ascontiguousarray(np.asarray(a, np.float32).T).astype(f16)

    per_core = [dict() for _ in range(NCORES)]
    shared = {}
    for s, (xv, wq, wk, wv, wo, w1, w2, w3, fn) in {
        "x": (x, wq_x, wk_x, wv_x, wo_x, w1_x, w2_x, w3_x, ffn_norm_x),
        "y": (y, wq_y, wk_y, wv_y, wo_y, w1_y, w2_y, w3_y, ffn_norm_y),
    }.items():
        xt = np.asarray(xv, np.float32).reshape(c.T, c.D).T  # [D, T]
        xt16 = np.ascontiguousarray(xt).astype(f16)
        shared[f"{s}T"] = xt16
        wqT = (np.asarray(wq, np.float32) * qscale[None, :]).T  # [D, D]
        wkT = (np.asarray(wk, np.float32) * nw[None, :]).T
        wvT = (np.asarray(wv, np.float32) * nw[None, :]).T
        woT = np.asarray(wo, np.float32).T                     # [Din, Dout]
        # pre-tiled FFN weights: partition-major so every chunk load is
        # one contiguous message per partition
        DT, FT = c.DT, c.FT
        w1T, w3T, w2T = t16(w1), t16(w3), t16(w2)
        shared[f"w1P_{s}"] = np.ascontiguousarray(
            w1T.reshape(DT, 128, c.FF).transpose(1, 2, 0))
        shared[f"w3P_{s}"] = np.ascontiguousarray(
            w3T.reshape(DT, 128, c.FF).transpose(1, 2, 0))
        shared[f"w2P_{s}"] = np.ascontiguousarray(
            w2T.reshape(2, FT // 2, 128, c.D).transpose(2, 0, 3, 1))
        shared[f"fnorm_{s}"] = np.ascontiguousarray(
            np.asarray(fn, np.float32).reshape(c.DT, 128).T)
        for r in range(NCORES):
            js = slice(r * c.NQ, (r + 1) * c.NQ)
            ts = slice(r * c.TC, (r + 1) * c.TC)
            per_core[r][f"wqT_{s}"] = np.ascontiguousarray(wqT[:, js]).astype(f16)
            per_core[r][f"wkT_{s}"] = np.ascontiguousarray(wkT[:, js]).astype(f16)
            per_core[r][f"wvT_{s}"] = np.ascontiguousarray(wvT[:, js]).astype(f16)
            per_core[r][f"woT_{s}"] = np.ascontiguousarray(woT[js, :]).astype(f16)
            per_core[r][f"res_{s}"] = np.ascontiguousarray(
                xt16[:, ts].reshape(c.DT, 128, c.TC).transpose(1, 0, 2))
    in_maps = []
    for r in range(NCORES):
        m = dict(shared)
        m.update(per_core[r])
        in_maps.append(m)
    return in_maps


def run(cfg, inputs, **kw):
    from concourse import bass_utils

    key = (cfg.B, cfg.S, cfg.D, cfg.H, cfg.HD, cfg.FF)
    if key not in _CACHE:
        _CACHE[key] = build(cfg)
    nc = _CACHE[key]
    in_maps = _prep_inputs(cfg, **{k: v for k, v in inputs.items()
                                   if k != "start_pos"})
    res = bass_utils.run_bass_kernel_spmd(
        nc, in_maps, core_ids=list(range(NCORES)), **kw)
    outs = []
    for s in ("x", "y"):
        cols = [res.results[r][f"out_{s}"] for r in range(NCORES)]
        full_t = np.concatenate(cols, axis=1)           # [D, T]
        outs.append(np.ascontiguousarray(full_t.T)
                    .reshape(cfg.B, cfg.S, cfg.D).astype(np.float32))
    return tuple(outs), res


def kernel(**inputs):
    (out_x, out_y), _ = run(FULL, inputs)
    return out_x, out_y
